# revision 3
# baseline (speedup 1.0000x reference)
"""Trainium2 Bass kernel: 4-layer MLP (784-512-512-512-10) + log_softmax.

Data-parallel over 8 NeuronCores: batch 65536 is split into 8 shards of
8192 rows; the ~1M-param weights are replicated on every core.

Layout choice: activations live on-chip transposed ([features, batch]) so
every layer's matmul is `out[of, nb] += W_lT[if_chunk, of_chunk].T @
h[if_chunk, nb]` with the feature chunks on partitions.  The input is
transposed/cast to bf16 on the host; matmuls run in bf16 with fp32 PSUM
accumulation; bias+ReLU is fused into one ScalarE activation per tile.
The last layer flips to `out[batch, 10]` (stationary = h3 chunk) so
log_softmax reduces along the free dim.
"""

from contextlib import ExitStack

import ml_dtypes
import numpy as np

import concourse.bass as bass  # noqa: F401  (registers AP machinery)
from concourse import bacc, mybir
from concourse.bass_utils import run_bass_kernel_spmd
from concourse.tile import TileContext

BF16 = mybir.dt.bfloat16
FP32 = mybir.dt.float32

N_CORES = 8
B = 65536
D0, H, C = 784, 512, 10
BC = B // N_CORES            # 8192 rows per core
NB = 512                     # batch tile (matmul moving free dim)
NCHUNK = BC // NB            # 16
K0F = D0 // 128              # 6 full 128-row contraction chunks in layer 1
K0R = D0 - K0F * 128         # 16 remainder rows
KH = H // 128                # 4 contraction chunks for hidden layers

_CACHED_NC = None


def build_nc():
    nc = bacc.Bacc(
        "TRN2",
        target_bir_lowering=False,
        debug=False,
        enable_asserts=True,
        num_devices=N_CORES,
    )
    xt_d = nc.declare_dram_parameter("xt", [D0, BC], BF16, isOutput=False)
    w1_d = nc.declare_dram_parameter("w1t", [D0, H], BF16, isOutput=False)
    w2_d = nc.declare_dram_parameter("w2t", [H, H], BF16, isOutput=False)
    w3_d = nc.declare_dram_parameter("w3t", [H, H], BF16, isOutput=False)
    w4_d = nc.declare_dram_parameter("w4t", [H, C], BF16, isOutput=False)
    b1_d = nc.declare_dram_parameter("b1", [H], FP32, isOutput=False)
    b2_d = nc.declare_dram_parameter("b2", [H], FP32, isOutput=False)
    b3_d = nc.declare_dram_parameter("b3", [H], FP32, isOutput=False)
    b4_d = nc.declare_dram_parameter("b4r", [128, C], FP32, isOutput=False)
    out_d = nc.declare_dram_parameter("out", [BC, C], FP32, isOutput=True)

    relu = mybir.ActivationFunctionType.Relu
    expf = mybir.ActivationFunctionType.Exp
    lnf = mybir.ActivationFunctionType.Ln

    with TileContext(nc) as tc, ExitStack() as ctx:
        consts = ctx.enter_context(tc.tile_pool(name="consts", bufs=1))
        xpool = ctx.enter_context(tc.tile_pool(name="xp", bufs=3))
        hpool = ctx.enter_context(tc.tile_pool(name="hp", bufs=2))
        opool = ctx.enter_context(tc.tile_pool(name="op", bufs=3))
        spool = ctx.enter_context(tc.tile_pool(name="sp", bufs=4))
        pbig = ctx.enter_context(tc.tile_pool(name="pbig", bufs=4, space="PSUM"))
        psml = ctx.enter_context(tc.tile_pool(name="psml", bufs=2, space="PSUM"))

        # Resident weights/biases, loaded once.
        w1 = consts.tile([128, K0F + 1, H], BF16, tag="w1")
        nc.sync.dma_start(
            w1[:, 0:K0F, :],
            w1_d[0 : K0F * 128, :].rearrange("(o p) n -> p o n", p=128),
        )
        nc.sync.dma_start(w1[0:K0R, K0F, :], w1_d[K0F * 128 : D0, :])
        w2 = consts.tile([128, KH, H], BF16, tag="w2")
        nc.sync.dma_start(w2[:], w2_d.rearrange("(o p) n -> p o n", p=128))
        w3 = consts.tile([128, KH, H], BF16, tag="w3")
        nc.sync.dma_start(w3[:], w3_d.rearrange("(o p) n -> p o n", p=128))
        w4 = consts.tile([128, KH, C], BF16, tag="w4")
        nc.sync.dma_start(w4[:], w4_d.rearrange("(o p) n -> p o n", p=128))
        b1s = consts.tile([128, KH], FP32, tag="b1")
        nc.sync.dma_start(b1s[:], b1_d.rearrange("(o p) -> p o", p=128))
        b2s = consts.tile([128, KH], FP32, tag="b2")
        nc.sync.dma_start(b2s[:], b2_d.rearrange("(o p) -> p o", p=128))
        b3s = consts.tile([128, KH], FP32, tag="b3")
        nc.sync.dma_start(b3s[:], b3_d.rearrange("(o p) -> p o", p=128))
        b4s = consts.tile([128, C], FP32, tag="b4")
        nc.sync.dma_start(b4s[:], b4_d[:])

        for cidx in range(NCHUNK):
            b0 = cidx * NB
            xt = xpool.tile([128, K0F + 1, NB], BF16, tag="xt")
            nc.sync.dma_start(
                xt[:, 0:K0F, :],
                xt_d[0 : K0F * 128, b0 : b0 + NB].rearrange("(o p) n -> p o n", p=128),
            )
            nc.sync.dma_start(xt[0:K0R, K0F, :], xt_d[K0F * 128 : D0, b0 : b0 + NB])

            # Layer 1: [784 -> 512], h1[of, nb] in bf16
            h1 = [hpool.tile([128, NB], BF16, tag=f"h1_{m}", name=f"h1_{m}") for m in range(KH)]
            for m in range(KH):
                ps = pbig.tile([128, NB], FP32, tag="ps")
                ms = slice(m * 128, (m + 1) * 128)
                for k in range(K0F):
                    nc.tensor.matmul(
                        ps[:], lhsT=w1[:, k, ms], rhs=xt[:, k, :],
                        start=(k == 0), stop=False,
                    )
                nc.tensor.matmul(
                    ps[:], lhsT=w1[0:K0R, K0F, ms], rhs=xt[0:K0R, K0F, :],
                    start=False, stop=True,
                )
                nc.scalar.activation(h1[m][:], ps[:], relu, bias=b1s[:, m : m + 1])

            # Layers 2, 3: [512 -> 512]
            h2 = [hpool.tile([128, NB], BF16, tag=f"h2_{m}", name=f"h2_{m}") for m in range(KH)]
            for m in range(KH):
                ps = pbig.tile([128, NB], FP32, tag="ps")
                ms = slice(m * 128, (m + 1) * 128)
                for k in range(KH):
                    nc.tensor.matmul(
                        ps[:], lhsT=w2[:, k, ms], rhs=h1[k][:],
                        start=(k == 0), stop=(k == KH - 1),
                    )
                nc.scalar.activation(h2[m][:], ps[:], relu, bias=b2s[:, m : m + 1])

            h3 = [hpool.tile([128, NB], BF16, tag=f"h3_{m}", name=f"h3_{m}") for m in range(KH)]
            for m in range(KH):
                ps = pbig.tile([128, NB], FP32, tag="ps")
                ms = slice(m * 128, (m + 1) * 128)
                for k in range(KH):
                    nc.tensor.matmul(
                        ps[:], lhsT=w3[:, k, ms], rhs=h2[k][:],
                        start=(k == 0), stop=(k == KH - 1),
                    )
                nc.scalar.activation(h3[m][:], ps[:], relu, bias=b3s[:, m : m + 1])

            # Layer 4 [512 -> 10] with output flipped to [batch, 10], then
            # log_softmax along the free dim.
            ot = opool.tile([128, KH, C], FP32, tag="ot")
            for m in range(KH):
                ps4 = psml.tile([128, C], FP32, tag="ps4")
                ms = slice(m * 128, (m + 1) * 128)
                for k in range(KH):
                    nc.tensor.matmul(
                        ps4[:], lhsT=h3[k][:, ms], rhs=w4[:, k, :],
                        start=(k == 0), stop=(k == KH - 1),
                    )
                lg = spool.tile([128, C], FP32, tag="lg")
                nc.vector.tensor_add(lg[:], ps4[:], b4s[:])
                negmax = spool.tile([128, 1], FP32, tag="negmax")
                nc.vector.tensor_reduce(
                    negmax[:], lg[:], axis=mybir.AxisListType.X,
                    op=mybir.AluOpType.max, negate=True,
                )
                esum = spool.tile([128, 1], FP32, tag="esum")
                etile = spool.tile([128, C], FP32, tag="etile")
                nc.scalar.activation(
                    etile[:], lg[:], expf, bias=negmax[:], accum_out=esum[:]
                )
                lns = spool.tile([128, 1], FP32, tag="lns")
                nc.scalar.activation(lns[:], esum[:], lnf)
                nbias = spool.tile([128, 1], FP32, tag="nbias")
                nc.vector.tensor_tensor(
                    nbias[:], negmax[:], lns[:], mybir.AluOpType.subtract
                )
                nc.vector.tensor_scalar_add(ot[:, m, :], lg[:], nbias[:])
            nc.sync.dma_start(
                out_d[b0 : b0 + NB, :].rearrange("(o p) n -> p o n", p=128), ot[:]
            )

    nc.compile()
    return nc


def _get_nc():
    global _CACHED_NC
    if _CACHED_NC is None:
        _CACHED_NC = build_nc()
    return _CACHED_NC


def make_in_maps(x, W1, b1, W2, b2, W3, b3, W4, b4):
    bf16 = ml_dtypes.bfloat16
    xbf = np.asarray(x).astype(bf16)
    common = {
        "w1t": np.ascontiguousarray(np.asarray(W1).T.astype(bf16)),
        "w2t": np.ascontiguousarray(np.asarray(W2).T.astype(bf16)),
        "w3t": np.ascontiguousarray(np.asarray(W3).T.astype(bf16)),
        "w4t": np.ascontiguousarray(np.asarray(W4).T.astype(bf16)),
        "b1": np.asarray(b1).astype(np.float32),
        "b2": np.asarray(b2).astype(np.float32),
        "b3": np.asarray(b3).astype(np.float32),
        "b4r": np.tile(np.asarray(b4).astype(np.float32)[None, :], (128, 1)),
    }
    in_maps = []
    for i in range(N_CORES):
        shard = np.ascontiguousarray(xbf[i * BC : (i + 1) * BC].T)  # [784, 8192]
        in_maps.append({"xt": shard, **common})
    return in_maps


def kernel(x, W1, b1, W2, b2, W3, b3, W4, b4):
    in_maps = make_in_maps(x, W1, b1, W2, b2, W3, b3, W4, b4)
    nc = _get_nc()
    res = run_bass_kernel_spmd(nc, in_maps, list(range(N_CORES)))
    out = np.concatenate(
        [res.results[i]["out"] for i in range(N_CORES)], axis=0
    ).astype(np.float32)
    return out


# revision 4
# speedup vs baseline: 1.3764x; 1.3764x over previous
"""Trainium2 Bass kernel: 4-layer MLP (784-512-512-512-10) + log_softmax.

Data-parallel over 8 NeuronCores: batch 65536 is split into 8 shards of
8192 rows; the ~1M-param weights are replicated on every core.

Layout choice: activations live on-chip transposed ([features, batch]) so
every layer's matmul is `out[of, nb] += W_lT[if_chunk, of_chunk].T @
h[if_chunk, nb]` with the feature chunks on partitions.  The input is
transposed/cast to bf16 on the host; matmuls run in bf16 with fp32 PSUM
accumulation.  bias+ReLU runs on VectorE (one fused tensor_scalar per
tile) so ScalarE only ever evaluates Exp — its activation table is
loaded once instead of thrashing between Relu/Exp/Ln.  The last layer
flips to `out[batch, 10]` (stationary = h3 chunk); log_softmax keeps
per-row -max and sum(exp) in persistent buffers and applies a single
batched Ln + broadcast-add at the end of the kernel.
"""

from contextlib import ExitStack

import ml_dtypes
import numpy as np

import concourse.bass as bass  # noqa: F401  (registers AP machinery)
from concourse import bacc, mybir
from concourse.bass_utils import run_bass_kernel_spmd
from concourse.tile import TileContext

BF16 = mybir.dt.bfloat16
FP32 = mybir.dt.float32

N_CORES = 8
B = 65536
D0, H, C = 784, 512, 10
BC = B // N_CORES            # 8192 rows per core
NB = 512                     # batch tile (matmul moving free dim)
NCHUNK = BC // NB            # 16
K0F = D0 // 128              # 6 full 128-row contraction chunks in layer 1
K0R = D0 - K0F * 128         # 16 remainder rows
KH = H // 128                # 4 contraction chunks for hidden layers
NRG = NCHUNK * KH            # 64 row-groups of 128 rows per core

_CACHED_NC = None


def build_nc():
    nc = bacc.Bacc(
        "TRN2",
        target_bir_lowering=False,
        debug=False,
        enable_asserts=False,
        num_devices=N_CORES,
    )
    xt_d = nc.declare_dram_parameter("xt", [D0, BC], BF16, isOutput=False)
    w1_d = nc.declare_dram_parameter("w1t", [D0, H], BF16, isOutput=False)
    w2_d = nc.declare_dram_parameter("w2t", [H, H], BF16, isOutput=False)
    w3_d = nc.declare_dram_parameter("w3t", [H, H], BF16, isOutput=False)
    w4_d = nc.declare_dram_parameter("w4t", [H, C], BF16, isOutput=False)
    b1_d = nc.declare_dram_parameter("b1", [H], FP32, isOutput=False)
    b2_d = nc.declare_dram_parameter("b2", [H], FP32, isOutput=False)
    b3_d = nc.declare_dram_parameter("b3", [H], FP32, isOutput=False)
    b4_d = nc.declare_dram_parameter("b4r", [128, C], FP32, isOutput=False)
    out_d = nc.declare_dram_parameter("out", [BC, C], FP32, isOutput=True)

    expf = mybir.ActivationFunctionType.Exp
    lnf = mybir.ActivationFunctionType.Ln
    add_op = mybir.AluOpType.add
    max_op = mybir.AluOpType.max
    sub_op = mybir.AluOpType.subtract

    with TileContext(nc) as tc, ExitStack() as ctx:
        consts = ctx.enter_context(tc.tile_pool(name="consts", bufs=1))
        xpool = ctx.enter_context(tc.tile_pool(name="xp", bufs=4))
        hpool = ctx.enter_context(tc.tile_pool(name="hp", bufs=2))
        spool = ctx.enter_context(tc.tile_pool(name="sp", bufs=4))
        pbig = ctx.enter_context(tc.tile_pool(name="pbig", bufs=4, space="PSUM"))
        psml = ctx.enter_context(tc.tile_pool(name="psml", bufs=2, space="PSUM"))

        # Resident weights/biases, loaded once.
        w1 = consts.tile([128, K0F + 1, H], BF16, tag="w1")
        nc.sync.dma_start(
            w1[:, 0:K0F, :],
            w1_d[0 : K0F * 128, :].rearrange("(o p) n -> p o n", p=128),
        )
        nc.sync.dma_start(w1[0:K0R, K0F, :], w1_d[K0F * 128 : D0, :])
        w2 = consts.tile([128, KH, H], BF16, tag="w2")
        nc.sync.dma_start(w2[:], w2_d.rearrange("(o p) n -> p o n", p=128))
        w3 = consts.tile([128, KH, H], BF16, tag="w3")
        nc.sync.dma_start(w3[:], w3_d.rearrange("(o p) n -> p o n", p=128))
        w4 = consts.tile([128, KH, C], BF16, tag="w4")
        nc.sync.dma_start(w4[:], w4_d.rearrange("(o p) n -> p o n", p=128))
        b1s = consts.tile([128, KH], FP32, tag="b1")
        nc.sync.dma_start(b1s[:], b1_d.rearrange("(o p) -> p o", p=128))
        b2s = consts.tile([128, KH], FP32, tag="b2")
        nc.sync.dma_start(b2s[:], b2_d.rearrange("(o p) -> p o", p=128))
        b3s = consts.tile([128, KH], FP32, tag="b3")
        nc.sync.dma_start(b3s[:], b3_d.rearrange("(o p) -> p o", p=128))
        b4s = consts.tile([128, C], FP32, tag="b4")
        nc.sync.dma_start(b4s[:], b4_d[:])

        # Persistent softmax state: logits, -max, sum(exp) for all 64
        # row-groups; combined in one batched pass at the end.
        logits_all = consts.tile([128, NRG, C], FP32, tag="logits_all")
        negmax_all = consts.tile([128, NRG], FP32, tag="negmax_all")
        esum_all = consts.tile([128, NRG], FP32, tag="esum_all")

        for cidx in range(NCHUNK):
            b0 = cidx * NB
            xt = xpool.tile([128, K0F + 1, NB], BF16, tag="xt")
            for k in range(K0F):
                nc.sync.dma_start(
                    xt[:, k, :],
                    xt_d[k * 128 : (k + 1) * 128, b0 : b0 + NB],
                )
            nc.sync.dma_start(xt[0:K0R, K0F, :], xt_d[K0F * 128 : D0, b0 : b0 + NB])

            # Layer 1: [784 -> 512], h1[of, nb] in bf16; bias+ReLU on DVE.
            h1 = [hpool.tile([128, NB], BF16, tag=f"h1_{m}", name=f"h1_{m}") for m in range(KH)]
            for m in range(KH):
                ps = pbig.tile([128, NB], FP32, tag="ps")
                ms = slice(m * 128, (m + 1) * 128)
                for k in range(K0F):
                    nc.tensor.matmul(
                        ps[:], lhsT=w1[:, k, ms], rhs=xt[:, k, :],
                        start=(k == 0), stop=False,
                    )
                nc.tensor.matmul(
                    ps[:], lhsT=w1[0:K0R, K0F, ms], rhs=xt[0:K0R, K0F, :],
                    start=False, stop=True,
                )
                nc.vector.tensor_scalar(
                    h1[m][:], ps[:], b1s[:, m : m + 1], 0.0, add_op, max_op
                )

            # Layers 2, 3: [512 -> 512]
            h2 = [hpool.tile([128, NB], BF16, tag=f"h2_{m}", name=f"h2_{m}") for m in range(KH)]
            for m in range(KH):
                ps = pbig.tile([128, NB], FP32, tag="ps")
                ms = slice(m * 128, (m + 1) * 128)
                for k in range(KH):
                    nc.tensor.matmul(
                        ps[:], lhsT=w2[:, k, ms], rhs=h1[k][:],
                        start=(k == 0), stop=(k == KH - 1),
                    )
                nc.vector.tensor_scalar(
                    h2[m][:], ps[:], b2s[:, m : m + 1], 0.0, add_op, max_op
                )

            h3 = [hpool.tile([128, NB], BF16, tag=f"h3_{m}", name=f"h3_{m}") for m in range(KH)]
            for m in range(KH):
                ps = pbig.tile([128, NB], FP32, tag="ps")
                ms = slice(m * 128, (m + 1) * 128)
                for k in range(KH):
                    nc.tensor.matmul(
                        ps[:], lhsT=w3[:, k, ms], rhs=h2[k][:],
                        start=(k == 0), stop=(k == KH - 1),
                    )
                nc.vector.tensor_scalar(
                    h3[m][:], ps[:], b3s[:, m : m + 1], 0.0, add_op, max_op
                )

            # Layer 4 [512 -> 10] with output flipped to [batch, 10].
            for m in range(KH):
                idx = cidx * KH + m
                ps4 = psml.tile([128, C], FP32, tag="ps4")
                ms = slice(m * 128, (m + 1) * 128)
                for k in range(KH):
                    nc.tensor.matmul(
                        ps4[:], lhsT=h3[k][:, ms], rhs=w4[:, k, :],
                        start=(k == 0), stop=(k == KH - 1),
                    )
                lg = logits_all[:, idx, :]
                nc.vector.tensor_add(lg, ps4[:], b4s[:])
                nc.vector.tensor_reduce(
                    negmax_all[:, idx : idx + 1], lg,
                    axis=mybir.AxisListType.X, op=max_op, negate=True,
                )
                etile = spool.tile([128, C], FP32, tag="etile")
                nc.scalar.activation(
                    etile[:], lg, expf,
                    bias=negmax_all[:, idx : idx + 1],
                    accum_out=esum_all[:, idx : idx + 1],
                )

        # Batched log_softmax epilogue: out = logits + (-max - ln(esum)).
        lns_all = consts.tile([128, NRG], FP32, tag="lns_all")
        nc.scalar.activation(lns_all[:], esum_all[:], lnf)
        nbias_all = consts.tile([128, NRG], FP32, tag="nbias_all")
        nc.vector.tensor_tensor(nbias_all[:], negmax_all[:], lns_all[:], sub_op)
        obuf = consts.tile([128, NRG, C], FP32, tag="obuf")
        nc.vector.tensor_tensor(
            obuf[:], logits_all[:],
            nbias_all[:, :, None].to_broadcast((128, NRG, C)), add_op,
        )
        nc.sync.dma_start(out_d.rearrange("(o p) n -> p o n", p=128), obuf[:])

    nc.compile()
    return nc


def _get_nc():
    global _CACHED_NC
    if _CACHED_NC is None:
        _CACHED_NC = build_nc()
    return _CACHED_NC


def make_in_maps(x, W1, b1, W2, b2, W3, b3, W4, b4):
    bf16 = ml_dtypes.bfloat16
    xbf = np.asarray(x).astype(bf16)
    common = {
        "w1t": np.ascontiguousarray(np.asarray(W1).T.astype(bf16)),
        "w2t": np.ascontiguousarray(np.asarray(W2).T.astype(bf16)),
        "w3t": np.ascontiguousarray(np.asarray(W3).T.astype(bf16)),
        "w4t": np.ascontiguousarray(np.asarray(W4).T.astype(bf16)),
        "b1": np.asarray(b1).astype(np.float32),
        "b2": np.asarray(b2).astype(np.float32),
        "b3": np.asarray(b3).astype(np.float32),
        "b4r": np.tile(np.asarray(b4).astype(np.float32)[None, :], (128, 1)),
    }
    in_maps = []
    for i in range(N_CORES):
        shard = np.ascontiguousarray(xbf[i * BC : (i + 1) * BC].T)  # [784, 8192]
        in_maps.append({"xt": shard, **common})
    return in_maps


def kernel(x, W1, b1, W2, b2, W3, b3, W4, b4):
    in_maps = make_in_maps(x, W1, b1, W2, b2, W3, b3, W4, b4)
    nc = _get_nc()
    res = run_bass_kernel_spmd(nc, in_maps, list(range(N_CORES)))
    out = np.concatenate(
        [res.results[i]["out"] for i in range(N_CORES)], axis=0
    ).astype(np.float32)
    return out
